# revision 25
# baseline (speedup 1.0000x reference)
"""GATv2 2-layer GNN kernel for Trainium2, distributed over 8 NeuronCores.

Strategy (dst-sharded graph parallel):
  - dst nodes sharded 8 ways (6250/core, padded to 49 blocks of 128).
  - Per layer: [node launch] xl = x@Wl, xr = x@Wr sharded per core (fp16),
    host gathers xl into a full [N,D] gather table; [edge launch] per core:
    dma_gather of xl[src] rows (fp16, 512B rows, int16 idx w/ lo/hi split),
    PE matmuls with one-hot incidence matrices for xr[dst] broadcast and
    the segment-softmax weighted aggregation, ACT leaky-relu/exp, DVE
    reduce + weighted multiply. Segment softmax computed without
    max-subtraction (scores are O(1), exp is safe).
  - Uniform program structure across cores (chunk counts padded to the max)
    so one SPMD program serves all 8 cores.
"""
import sys

sys.path.insert(0, '/opt/trn_rl_repo')

import numpy as np
import ml_dtypes

import concourse.bass as bass
import concourse.mybir as mybir
from concourse import bacc
from concourse.tile import TileContext
from concourse import library_config

F32 = mybir.dt.float32
F16 = mybir.dt.float16
FP8 = mybir.dt.float8e4
I16 = mybir.dt.int16
NPF8 = mybir.dt.np(FP8)
FP8_ONE = np.float32(1.0).astype(NPF8).view(np.uint8).item()

N = 50000
D = 256
NH = 8
CW = 32
NCORES = 8
NEG = 0.2
SPLIT = 32768

LAST_RUN_INFO = {}


# --------------------------------------------------------------------------
# Host-side planning: block assignment, chunking, incidence/index buffers
# --------------------------------------------------------------------------

def _plan(src, dst, n, ncores, nblk, split):
    """Build the uniform per-core execution plan.

    Returns dict with structure fields (identical across cores) and
    per-core data buffers.
    """
    own = n // ncores
    ownpad = nblk * 128
    rng = np.random.default_rng(0)

    per_core = []
    maxL = maxH = 0
    for c in range(ncores):
        lo_b, hi_b = c * own, (c + 1) * own
        m = (dst >= lo_b) & (dst < hi_b)
        es = src[m].astype(np.int64)
        ed = (dst[m] - lo_b).astype(np.int64)
        deg = np.bincount(ed, minlength=own)

        # greedy balance nodes into nblk blocks of <=128 by total degree
        order = np.argsort(-deg, kind='stable')
        bl_load = np.zeros(nblk, np.int64)
        bl_cnt = np.zeros(nblk, np.int64)
        node_block = np.empty(own, np.int64)
        node_slot = np.empty(own, np.int64)
        for nd in order:
            avail = bl_cnt < 128
            b = int(np.flatnonzero(avail)[np.argmin(bl_load[avail])])
            node_block[nd] = b
            node_slot[nd] = bl_cnt[b]
            bl_cnt[b] += 1
            bl_load[b] += deg[nd]

        # slot permutation: perm[b*128+s] = local node id (or -1 for pad)
        perm = np.full(ownpad, -1, np.int64)
        perm[node_block * 128 + node_slot] = np.arange(own)

        # per-edge block/slot
        e_blk = node_block[ed]
        e_slot = node_slot[ed]
        e_lo = es < split

        # dummy edges for pad slots (keeps den > 0); src node 0 is lo
        pad_pos = np.flatnonzero(perm < 0)
        if len(pad_pos):
            es = np.concatenate([es, np.zeros(len(pad_pos), np.int64)])
            e_blk = np.concatenate([e_blk, pad_pos // 128])
            e_slot = np.concatenate([e_slot, pad_pos % 128])
            e_lo = np.concatenate([e_lo, np.ones(len(pad_pos), bool)])

        lo_cnt = np.bincount(e_blk[e_lo], minlength=nblk)
        hi_cnt = np.bincount(e_blk[~e_lo], minlength=nblk)
        maxL = max(maxL, int(np.ceil(lo_cnt.max() / 128)))
        maxH = max(maxH, int(np.ceil(max(hi_cnt.max(), 1) / 128)))
        per_core.append((es, e_blk, e_slot, e_lo, perm))

    L, H = maxL, maxH
    cpb = L + H                      # chunks per block
    nch = nblk * cpb                 # chunks per core

    # supertile structure (identical for every core):
    # per block: lo chunks grouped by <=STL, then hi chunks grouped by <=STL
    STL = 6
    sts = []     # (blk, half, chunk0, stlen, iccol0)
    iccol = 0
    for b in range(nblk):
        for half, cnt, base in ((0, L, b * cpb), (1, H, b * cpb + L)):
            j = 0
            while j < cnt:
                sl = min(STL, cnt - j)
                sts.append((b, half, base + j, sl, iccol))
                iccol += 8 * sl
                j += sl
    icols = iccol

    cores = []
    for c in range(ncores):
        es, e_blk, e_slot, e_lo, perm = per_core[c]
        src_adj = np.zeros((nch, 128), np.int16)
        dst_loc = np.zeros((nch, 128), np.int16)
        valid = np.zeros((nch, 128), bool)
        for b in range(nblk):
            for half, cnt, base in ((0, L, b * cpb), (1, H, b * cpb + L)):
                sel = np.flatnonzero((e_blk == b) & (e_lo == (half == 0)))
                k = len(sel)
                assert k <= cnt * 128, (c, b, half, k)
                flat_s = np.zeros(cnt * 128, np.int64)
                flat_d = np.zeros(cnt * 128, np.int64)
                flat_v = np.zeros(cnt * 128, bool)
                flat_s[:k] = es[sel] - (split if half else 0)
                flat_d[:k] = e_slot[sel]
                flat_v[:k] = True
                src_adj[base:base + cnt] = flat_s.reshape(cnt, 128)
                dst_loc[base:base + cnt] = flat_d.reshape(cnt, 128)
                valid[base:base + cnt] = flat_v.reshape(cnt, 128)

        # incidence matrices in fp8 (exact one-hot), packed [AT_ch | A_ch]
        AAT = np.zeros((128, nch * 256), np.uint8)
        ch_i = np.repeat(np.arange(nch), 128)
        e_i = np.tile(np.arange(128), nch)
        v = valid.ravel()
        AAT[e_i[v], ch_i[v] * 256 + 128 + dst_loc.ravel()[v]] = FP8_ONE   # A
        AAT[dst_loc.ravel()[v], ch_i[v] * 256 + e_i[v]] = FP8_ONE         # AT

        # gather index buffer: per supertile, positions wrapped in 16 rows
        idxw = np.zeros((16, icols), np.int16)
        for (b, half, c0, sl, ic0) in sts:
            vals = src_adj[c0:c0 + sl].ravel()       # 128*sl positions
            pos = np.arange(128 * sl)
            idxw[pos % 16, ic0 + pos // 16] = vals
        idxw = np.tile(idxw, (8, 1))                 # replicate to 128 parts

        cores.append(dict(perm=perm, AATg=AAT.view(NPF8), idxw=idxw))

    return dict(n=n, ncores=ncores, own=own, nblk=nblk, ownpad=ownpad,
                split=split, L=L, H=H, cpb=cpb, nch=nch, icols=icols,
                stl=STL, sts=sts, cores=cores)


# --------------------------------------------------------------------------
# Bass program builders
# --------------------------------------------------------------------------

def _build_node(mpad, d=D):
    """xT [d, mpad] f16, Wl/Wr [d, d] f16 -> xl/xr [mpad, d] f16."""
    nc = bacc.Bacc('TRN2', target_bir_lowering=False, debug=False)
    xT = nc.dram_tensor("xT", [d, mpad], F16, kind="ExternalInput")
    Wl = nc.dram_tensor("Wl", [d, d], F16, kind="ExternalInput")
    Wr = nc.dram_tensor("Wr", [d, d], F16, kind="ExternalInput")
    xl = nc.dram_tensor("xl", [mpad, d], F16, kind="ExternalOutput")
    xr = nc.dram_tensor("xr", [mpad, d], F16, kind="ExternalOutput")
    kh = d // 128
    with TileContext(nc) as tc:
        with (tc.tile_pool(name="w", bufs=1) as wp,
              tc.tile_pool(name="io", bufs=6) as iop,
              tc.tile_pool(name="ps", bufs=4, space="PSUM") as pp):
            wl_t = wp.tile([128, kh, d], F16, tag="wl")
            wr_t = wp.tile([128, kh, d], F16, tag="wr")
            nc.sync.dma_start(out=wl_t[:], in_=Wl[:].rearrange("(k p) n -> p k n", p=128))
            nc.sync.dma_start(out=wr_t[:], in_=Wr[:].rearrange("(k p) n -> p k n", p=128))
            # batch tiles in groups of 8: the per-DMA HWDGE queue cost
            # (~625ns) dominates this launch, so load/store 8 tiles per DMA
            G = 8
            nt = mpad // 128
            for t0 in range(0, nt, G):
                g = min(G, nt - t0)
                lh = iop.tile([128, kh, G * 128], F16, tag="lh")
                nc.sync.dma_start(
                    out=lh[:, :, 0:g * 128],
                    in_=xT[:, t0 * 128:(t0 + g) * 128].rearrange(
                        "(k p) m -> p k m", p=128))
                for w_t, out_d, tg in ((wl_t, xl, "ol"), (wr_t, xr, "orr")):
                    o = iop.tile([128, G, d], F16, tag=tg)
                    for j in range(g):
                        ps = pp.tile([128, d], F32, tag="ps")
                        for k in range(kh):
                            nc.tensor.matmul(
                                ps[:], lh[:, k, j * 128:(j + 1) * 128],
                                w_t[:, k, :], start=(k == 0), stop=(k == kh - 1))
                        nc.scalar.copy(out=o[:, j, :], in_=ps[:])
                    nc.sync.dma_start(
                        out=out_d[t0 * 128:(t0 + g) * 128, :].rearrange(
                            "(t p) d -> p t d", p=128),
                        in_=o[:, 0:g, :])
    nc.compile()
    return nc


def _build_edge(plan, elu, out_f32, sim_safe=False, use_bias=True):
    """Edge-phase program for one layer (uniform across cores)."""
    n, nblk, split = plan['n'], plan['nblk'], plan['split']
    nch, icols, sts, cpb = plan['nch'], plan['icols'], plan['sts'], plan['cpb']
    ownpad = plan['ownpad']
    OD = F32 if out_f32 else F16
    # Prelu == leaky-relu with runtime alpha; lives in the same activation
    # table set as Exp (exp_and_others), so no table reloads. (Lrelu is
    # broken on HW: ignores alpha.)
    act_f = (mybir.ActivationFunctionType.Relu if sim_safe
             else mybir.ActivationFunctionType.Prelu)

    nc = bacc.Bacc('TRN2', target_bir_lowering=False, debug=False)
    xlf = nc.dram_tensor("xlf", [n, D], F16, kind="ExternalInput")
    xro = nc.dram_tensor("xro", [ownpad, D], F16, kind="ExternalInput")
    AATg = nc.dram_tensor("AATg", [128, nch * 256], FP8, kind="ExternalInput")
    idxw = nc.dram_tensor("idxw", [128, icols], I16, kind="ExternalInput")
    attb = nc.dram_tensor("attb", [128, D], F16, kind="ExternalInput")
    biasb = nc.dram_tensor("biasb", [128, D], F16, kind="ExternalInput")
    ident = nc.dram_tensor("ident", [128, 128], FP8, kind="ExternalInput")
    outd = nc.dram_tensor("outd", [ownpad, D], OD, kind="ExternalOutput")

    from contextlib import ExitStack
    with TileContext(nc) as tc, ExitStack() as stack:
        nc.gpsimd.load_library(library_config.mlp)
        # one shared register per distinct gather size (to_reg would burn
        # a fresh register per call under Tile and exhaust the pool)
        nregs = {}
        for v in sorted({128 * s[3] for s in sts}):
            r = stack.enter_context(nc.gpsimd.register(f"nidx{v}"))
            nc.gpsimd.reg_mov(r, v)
            nregs[v] = r
        with (tc.tile_pool(name="const", bufs=1) as cp,
              tc.tile_pool(name="ab", bufs=7) as abp,
              tc.tile_pool(name="gt", bufs=7) as gtp,
              tc.tile_pool(name="mid", bufs=7) as mp,
              tc.tile_pool(name="ep", bufs=4) as epp,
              tc.tile_pool(name="pss", bufs=2, space="PSUM") as psp,
              tc.tile_pool(name="psb", bufs=2, space="PSUM") as pbp):
            att_sb = cp.tile([128, D], F16, tag="att")
            nc.sync.dma_start(out=att_sb[:], in_=attb[:])
            if use_bias:
                bias_sb = cp.tile([128, D], F16, tag="bias")
                nc.sync.dma_start(out=bias_sb[:], in_=biasb[:])
            id_sb = cp.tile([128, 128], FP8, tag="id")
            nc.sync.dma_start(out=id_sb[:], in_=ident[:])
            idx_sb = cp.tile([128, icols], I16, tag="idx")
            nc.sync.dma_start(out=idx_sb[:], in_=idxw[:])
            xr_sb = cp.tile([128, nblk, D], F16, tag="xr")
            nc.sync.dma_start(
                out=xr_sb[:], in_=xro[:].rearrange("(b p) d -> p b d", p=128))

            STL = plan['stl']
            ps_blk = None
            for si, (b, hf, c0, sl, ic0) in enumerate(sts):
                first_of_blk = (si == 0) or (sts[si - 1][0] != b)
                last_of_blk = (si == len(sts) - 1) or (sts[si + 1][0] != b)
                if first_of_blk:
                    ps_blk = pbp.tile([128, D + 8], F32, tag="psb")

                # gather xl rows for these sl chunks (all same half)
                XL = gtp.tile([128, STL, D], F16, tag="xl")
                src_ap = xlf[0:split, :] if hf == 0 else xlf[split:n, :]
                nc.gpsimd.dma_gather(
                    out_ap=XL[:, 0:sl, :],
                    in_ap=src_ap,
                    idxs_ap=idx_sb[:, ic0:ic0 + 8 * sl],
                    num_idxs=128 * sl,
                    num_idxs_reg=nregs[128 * sl],
                    elem_size=D,
                )

                # packed [AT_ch | A_ch] incidence slices, one DMA
                aat = abp.tile([128, STL * 256], FP8, tag="aat")
                nc.sync.dma_start(out=aat[:, 0:sl * 256],
                                  in_=AATg[:, c0 * 256:(c0 + sl) * 256])

                # s = xr[dst] + xl[src] in psum
                ps = psp.tile([128, STL, D], F32, tag="pss")
                for j in range(sl):
                    # start=True only on the first matmul touching each
                    # 2KB psum bank (2 chunks/bank): pending-zero is
                    # bank-granular.
                    nc.tensor.matmul(ps[:, j, :], aat[:, j * 256:j * 256 + 128],
                                     xr_sb[:, b, :], start=(j % 2 == 0),
                                     stop=False, skip_group_check=True)
                for j0 in range(0, sl, 2):
                    j1 = min(j0 + 2, sl)
                    nc.tensor.matmul(ps[:, j0:j1, :], id_sb[:], XL[:, j0:j1, :],
                                     start=False, stop=True,
                                     skip_group_check=True)

                # L = leaky_relu(s) ; m = L * att ; e = per-head tree-sum
                Lt = mp.tile([128, STL, D], F16, tag="L")
                nc.scalar.activation(out=Lt[:, 0:sl, :], in_=ps[:, 0:sl, :],
                                     func=act_f, alpha=NEG)
                mt = mp.tile([128, STL, D], F16, tag="m")
                nc.vector.tensor_tensor(
                    out=mt[:, 0:sl, :], in0=Lt[:, 0:sl, :],
                    in1=att_sb[:].unsqueeze(1).broadcast_to([128, sl, D]),
                    op=mybir.AluOpType.mult)
                cur, curw = mt[:, 0:sl, :].rearrange("p s (h w) -> p s h w",
                                                     h=NH), CW
                while curw > 2:
                    hw = curw // 2
                    nt = mp.tile([128, STL, NH, hw], F16, tag=f"tr{hw}")
                    nc.vector.tensor_tensor(
                        out=nt[:, 0:sl], in0=cur[:, :, :, 0:hw],
                        in1=cur[:, :, :, hw:curw], op=mybir.AluOpType.add)
                    cur, curw = nt[:, 0:sl], hw
                et = mp.tile([128, STL, NH], F32, tag="e")
                nc.vector.tensor_tensor(
                    out=et[:, 0:sl, :].unsqueeze(3),
                    in0=cur[:, :, :, 0:1], in1=cur[:, :, :, 1:2],
                    op=mybir.AluOpType.add)

                # w = exp(e) channel-expanded on ACT (stride-0 input) so the
                # y multiply has two dense operands -> DVE 2x mode.
                ww = mp.tile([128, STL, D], F16, tag="ww")
                nc.scalar.activation(
                    out=ww[:, 0:sl, :].rearrange("p s (h w) -> p s h w", h=NH),
                    in_=et[:, 0:sl, :].unsqueeze(3).broadcast_to(
                        [128, sl, NH, CW]),
                    func=mybir.ActivationFunctionType.Exp)
                yt = mp.tile([128, STL, D], F16, tag="y")
                nc.vector.tensor_tensor(
                    out=yt[:, 0:sl, :], in0=XL[:, 0:sl, :],
                    in1=ww[:, 0:sl, :], op=mybir.AluOpType.mult)

                # aggregate: psb[:, 0:D] += A_ch^T @ y ; psb[:, D:] += A^T @ w
                for j in range(sl):
                    a_j = aat[:, j * 256 + 128:(j + 1) * 256]
                    nc.tensor.matmul(ps_blk[:, 0:D], a_j, yt[:, j, :],
                                     start=(first_of_blk and j == 0),
                                     stop=False, skip_group_check=True)
                    nc.tensor.matmul(
                        ps_blk[:, D:D + 8], a_j,
                        ww[:, j, :].rearrange("p (h w) -> p h w", h=NH)[:, :, 0],
                        start=False,
                        stop=(last_of_blk and j == sl - 1),
                        skip_group_check=True)

                if last_of_blk:
                    rec = epp.tile([128, NH], F32, tag="rec")
                    nc.vector.reciprocal(rec[:], ps_blk[:, D:D + 8])
                    o1 = epp.tile([128, D], F16 if (elu or use_bias) else OD,
                                  tag="o1")
                    nc.vector.tensor_tensor(
                        out=o1[:].rearrange("p (h w) -> p h w", h=NH),
                        in0=ps_blk[:, 0:D].rearrange("p (h w) -> p h w", h=NH),
                        in1=rec[:].unsqueeze(2).broadcast_to([128, NH, CW]),
                        op=mybir.AluOpType.mult)
                    if use_bias:
                        o2 = epp.tile([128, D], F16 if elu else OD, tag="o2")
                        nc.vector.tensor_tensor(out=o2[:], in0=o1[:],
                                                in1=bias_sb[:],
                                                op=mybir.AluOpType.add)
                    else:
                        o2 = o1
                    if elu:
                        ex = epp.tile([128, D], F16, tag="ex")
                        nc.scalar.activation(out=ex[:], in_=o2[:],
                                             func=mybir.ActivationFunctionType.Exp)
                        # min(exp(x),1)-1  == exp(min(x,0))-1
                        t1 = epp.tile([128, D], F16, tag="t1")
                        nc.vector.tensor_scalar(out=t1[:], in0=ex[:],
                                                scalar1=1.0, scalar2=-1.0,
                                                op0=mybir.AluOpType.min,
                                                op1=mybir.AluOpType.add)
                        t2 = epp.tile([128, D], F16, tag="t2")
                        nc.vector.tensor_scalar(out=t2[:], in0=o2[:],
                                                scalar1=0.0, scalar2=None,
                                                op0=mybir.AluOpType.max)
                        ho = epp.tile([128, D], OD, tag="ho")
                        nc.vector.tensor_tensor(out=ho[:], in0=t1[:], in1=t2[:],
                                                op=mybir.AluOpType.add)
                    else:
                        ho = o2
                    nc.sync.dma_start(out=outd[b * 128:(b + 1) * 128, :], in_=ho[:])
    nc.compile()
    return nc


# --------------------------------------------------------------------------
# Runner
# --------------------------------------------------------------------------

RUNNER_OVERRIDE = [None]  # test hook: set to fn(nc, in_maps) -> list[dict]


def _run(nc, in_maps, trace=False):
    if RUNNER_OVERRIDE[0] is not None:
        return RUNNER_OVERRIDE[0](nc, in_maps)
    from concourse.bass_utils import run_bass_kernel_spmd
    res = run_bass_kernel_spmd(nc, in_maps, core_ids=list(range(len(in_maps))),
                               trace=trace)
    if res.exec_time_ns is not None:
        LAST_RUN_INFO.setdefault('exec_ns', []).append(res.exec_time_ns)
    return res.results


def _layer(plan, nodes_feat, Wl, Wr, att, bias, edge_nc, node_nc, trace):
    """Run one GAT layer. nodes_feat [N, D] f32/f16; returns [N, D] f32."""
    n, ncores, ownpad, own = plan['n'], plan['ncores'], plan['ownpad'], plan['own']
    f16 = np.float16

    Wl16, Wr16 = Wl.astype(f16), Wr.astype(f16)
    xTs, perms = [], []
    for c in range(ncores):
        perm = plan['cores'][c]['perm']
        shard = nodes_feat[c * own:(c + 1) * own]
        xT = np.zeros((D, ownpad), f16)
        valid = perm >= 0
        xT[:, valid] = shard[perm[valid]].T.astype(f16)
        xTs.append(xT)
        perms.append(perm)

    node_res = _run(node_nc,
                    [dict(xT=xTs[c], Wl=Wl16, Wr=Wr16) for c in range(ncores)],
                    trace)

    xl_full = np.zeros((n, D), f16)
    for c in range(ncores):
        perm = perms[c]
        valid = perm >= 0
        xl_full[c * own + perm[valid]] = node_res[c]['xl'][valid]

    attb = np.tile(att.reshape(1, -1), (128, 1)).astype(f16)
    biasb = np.tile(bias.reshape(1, -1), (128, 1)).astype(f16)
    identity = np.eye(128, dtype=np.float32).astype(NPF8)

    in_maps = []
    for c in range(ncores):
        cd = plan['cores'][c]
        in_maps.append(dict(xlf=xl_full, xro=node_res[c]['xr'],
                            AATg=cd['AATg'], idxw=cd['idxw'],
                            attb=attb, biasb=biasb, ident=identity))
    edge_res = _run(edge_nc, in_maps, trace)
    return edge_res, perms


_PLAN_CACHE = {}
_PROG_CACHE = {}


def kernel(x, edges_idx, Wl1, Wr1, att1, b1, Wl2, Wr2, att2, b2,
           _trace=False, _sim_safe=False):
    x = np.asarray(x)
    edges_idx = np.asarray(edges_idx)
    LAST_RUN_INFO.clear()

    nblk = (N // NCORES + 127) // 128
    ek = edges_idx.tobytes()[:64]  # cheap cache key for repeated calls
    key = (edges_idx.shape[1], hash(ek))
    if key not in _PLAN_CACHE:
        loop = np.arange(N, dtype=np.int64)
        src = np.concatenate([edges_idx[0].astype(np.int64), loop])
        dst = np.concatenate([edges_idx[1].astype(np.int64), loop])
        _PLAN_CACHE[key] = _plan(src, dst, N, NCORES, nblk, SPLIT)
    plan = _PLAN_CACHE[key]

    ub1 = bool(np.abs(np.asarray(b1)).max() > 0)
    ub2 = bool(np.abs(np.asarray(b2)).max() > 0)
    pkey = (plan['nch'], _sim_safe, ub1, ub2)
    if pkey not in _PROG_CACHE:
        _PROG_CACHE[pkey] = (
            _build_node(plan['ownpad']),
            _build_edge(plan, elu=True, out_f32=False, sim_safe=_sim_safe,
                        use_bias=ub1),
            _build_edge(plan, elu=False, out_f32=True, sim_safe=_sim_safe,
                        use_bias=ub2),
        )
    node_nc, edge1_nc, edge2_nc = _PROG_CACHE[pkey]

    att1f = np.asarray(att1).reshape(-1)
    att2f = np.asarray(att2).reshape(-1)

    # layer 1
    e1, perms = _layer(plan, np.asarray(x, np.float32), np.asarray(Wl1),
                       np.asarray(Wr1), att1f, np.asarray(b1), edge1_nc,
                       node_nc, _trace)
    own = plan['own']
    h = np.zeros((N, D), np.float16)
    for c in range(NCORES):
        perm = perms[c]
        valid = perm >= 0
        h[c * own + perm[valid]] = e1[c]['outd'][valid]

    # layer 2
    e2, perms = _layer(plan, h.astype(np.float32), np.asarray(Wl2),
                       np.asarray(Wr2), att2f, np.asarray(b2), edge2_nc,
                       node_nc, _trace)
    out = np.zeros((N, D), np.float32)
    for c in range(NCORES):
        perm = perms[c]
        valid = perm >= 0
        out[c * own + perm[valid]] = e2[c]['outd'][valid]
    return out
